# revision 1
# baseline (speedup 1.0000x reference)
"""Contrastive (InfoNCE-style symmetric) loss on 8 trn2 NeuronCores.

Reference math (B=4096, D=1024, fp32):
    xn = x / max(||x_i||, eps);  yn = y / max(||y_j||, eps)
    S[i,j] = xn_i . yn_j ;  E = exp(S/tau)
    extra = B*eps + eps
    row_denom_i = sum_j E[i,j] + extra ; col_denom_j = sum_i E[i,j] + extra
    loss = -1/(2B) * ( 2*sum_i S_ii/tau - sum_i ln(row_denom_i)
                       - sum_j ln(col_denom_j) )

Sharding: batch dim of x is split across the 8 cores (512 rows each); every
core holds the full y (transposed). Each core computes its [4096, 512] block
of S^T on TensorE (j on partitions, local i on free), normalization folded in
afterwards: tensor_tensor multiply by 1/||x_i|| (partition-broadcast of the
local rx vector) while draining PSUM, then ACT Exp with per-partition scale
1/(tau*||y_j||). The same ACT pass emits per-block column partial sums via
accum_out. Row denominators accumulate on TensorE as ones^T @ E, lagging the
exp stage. One AllGather shares the pre-scaled reciprocal y norms (the
post-gather path is a single DMA); the column partials + scalar partial
terms are combined by two AllReduces (first half mid-stream, second half at
the end) so most of the collective latency overlaps the matmul stream. A
tiny dummy AllGather issued at kernel start absorbs the one-time collective
entry barrier. A burst of dummy warm-up matmuls engages the PE clock
un-throttle while the input DMAs are in flight.

Inputs are cast to bf16 on the host (matmul operand dtype). Loss error stays
~1e-5 relative: per-element rounding noise averages out across the 2*4096
log terms and PSUM accumulation is fp32 throughout.
"""
import numpy as np
import ml_dtypes

import concourse.bacc as bacc
import concourse.mybir as mybir
import concourse.tile as tile
from concourse.bass_utils import run_bass_kernel_spmd

AF = mybir.ActivationFunctionType
ALU = mybir.AluOpType
BF16 = mybir.dt.bfloat16
F32 = mybir.dt.float32

B = 4096
D = 1024
N_CORES = 8
BL = B // N_CORES          # 512 local x rows
TAU = 0.07
EPS = 1e-6
EXTRA = B * EPS + EPS
COEF = -1.0 / (2.0 * B)

ND = D // 128              # 8 contraction chunks
NJB = B // 128             # 32 j-blocks (PSUM partition dim)
N_WARM = 16                # dummy matmuls to warm the PE clock gate
TB_BUFS = 24               # PSUM-drain tiles: PE run-ahead of the AG latency
LAG = 22                   # row-sum matmul lag behind the exp stage

_cache: dict = {}


def _build():
    nc = bacc.Bacc("TRN2", target_bir_lowering=False, debug=False,
                   num_devices=N_CORES)

    xT = nc.dram_tensor("xT", [D, BL], BF16, kind="ExternalInput")
    yT = nc.dram_tensor("yT", [D, B], BF16, kind="ExternalInput")
    yTo = nc.dram_tensor("yTown", [D, BL], BF16, kind="ExternalInput")
    loss_out = nc.dram_tensor("loss", [1, 1], F32, kind="ExternalOutput")

    rg = [list(range(N_CORES))]

    with tile.TileContext(nc) as tc:
        with (
            tc.tile_pool(name="res", bufs=1) as res,
            tc.tile_pool(name="tmp", bufs=3) as tmp,
            tc.tile_pool(name="tblk", bufs=TB_BUFS) as tpool,
            tc.tile_pool(name="eblk", bufs=NJB) as epool,
            tc.tile_pool(name="pg", bufs=4, space="PSUM") as pg,
            tc.tile_pool(name="pa", bufs=2, space="PSUM") as pa,
            tc.tile_pool(name="pw", bufs=1, space="PSUM") as pw,
            tc.tile_pool(name="prow", bufs=1, space="PSUM") as prow,
            tc.tile_pool(name="dram", bufs=1, space="DRAM") as dr,
        ):
            # ---- PE warm-up: dummy matmuls while input DMAs fly ----
            wsrc = res.tile([128, 512], BF16, name="wsrc")
            nc.vector.memset(wsrc[:], 0.125)
            wp = pw.tile([128, 512], F32, tag="pw", name="wp")
            for _ in range(N_WARM):
                nc.tensor.matmul(wp[:], wsrc[:, 0:128], wsrc[:],
                                 start=True, stop=True, skip_group_check=True)

            # ---- input DMAs ----
            xts = []
            for d in range(ND):
                t = res.tile([128, BL], BF16, tag=f"xt{d}", name=f"xt{d}")
                nc.sync.dma_start(t[:], xT[d * 128:(d + 1) * 128, :])
                xts.append(t)
            ytos = []
            for d in range(ND):
                t = res.tile([128, BL], BF16, tag=f"yo{d}", name=f"yo{d}")
                nc.sync.dma_start(t[:], yTo[d * 128:(d + 1) * 128, :])
                ytos.append(t)
            yts = {}
            for g2 in range(2):
                for d in range(ND):
                    t = res.tile([128, 2048], BF16, tag=f"yt{g2}_{d}",
                                 name=f"yt{g2}_{d}")
                    nc.sync.dma_start(
                        t[:],
                        yT[d * 128:(d + 1) * 128, g2 * 2048:(g2 + 1) * 2048])
                    yts[(g2, d)] = t

            ones_bf = res.tile([128, 1], BF16, name="ones_bf")
            nc.vector.memset(ones_bf[:], 1.0)
            ones_f = res.tile([128, 1], F32, name="ones_f")
            nc.vector.memset(ones_f[:], 1.0)

            # ---- ||x||^2 first (rx gates the PSUM-drain TT stream) ----
            p_nx = pa.tile([1, 512], F32, tag="pa", name="p_nx")
            p_ny = pa.tile([1, 512], F32, tag="pa", name="p_ny")
            for d in range(ND):
                sq = tmp.tile([128, 512], BF16, tag="sq", name="sq")
                nc.vector.tensor_mul(sq[:], xts[d][:], xts[d][:])
                nc.tensor.matmul(p_nx[:], ones_bf[:], sq[:],
                                 start=(d == 0), stop=(d == ND - 1))
            for d in range(ND):
                sq2 = tmp.tile([128, 512], BF16, tag="sq", name="sq2")
                nc.vector.tensor_mul(sq2[:], ytos[d][:], ytos[d][:])
                nc.tensor.matmul(p_ny[:], ones_bf[:], sq2[:],
                                 start=(d == 0), stop=(d == ND - 1))

            # ---- rx chain (PE-critical) ----
            nx = tmp.tile([1, 512], F32, tag="v", name="nx")
            nc.scalar.activation(nx[:], p_nx[:], AF.Sqrt)
            nxm = tmp.tile([1, 512], F32, tag="v", name="nxm")
            nc.vector.tensor_scalar_max(nxm[:], nx[:], EPS)
            rx = res.tile([1, 512], F32, name="rx")
            nc.vector.reciprocal(rx[:], nxm[:])
            rx_d = dr.tile([BL], F32, name="rx_d")
            nc.gpsimd.dma_start(rx_d[:], rx[:])
            rx_b = res.tile([128, 512], F32, name="rx_b")
            nc.gpsimd.dma_start(
                rx_b[:],
                rx_d[:].rearrange("(o a) -> o a", o=1).broadcast_to([128, BL]))

            # ---- AG chain: rys_own = 1/(tau*max(||y_own||,eps)) ----
            ny = tmp.tile([1, 512], F32, tag="v", name="ny")
            nc.scalar.activation(ny[:], p_ny[:], AF.Sqrt)
            nym = res.tile([1, 512], F32, name="nym")
            nc.vector.tensor_scalar_max(nym[:], ny[:], EPS)
            ryo = res.tile([1, 512], F32, name="ryo")
            nc.vector.reciprocal(ryo[:], nym[:])
            rys_own = tmp.tile([1, 512], F32, tag="v", name="rys_own")
            nc.vector.tensor_scalar_mul(rys_own[:], ryo[:], 1.0 / TAU)
            ag_in = dr.tile([BL], F32, name="ag_in")
            nc.gpsimd.dma_start(ag_in[:], rys_own[:])
            ag_out = dr.tile([B], F32, name="ag_out")
            nc.gpsimd.collective_compute(
                "AllGather", ALU.bypass, replica_groups=rg,
                ins=[ag_in.opt()], outs=[ag_out.opt()])
            ry_scl = res.tile([128, 32], F32, name="ry_scl")
            for k in range(N_CORES):
                nc.gpsimd.dma_start(
                    ry_scl[:, 4 * k:4 * k + 4],
                    ag_out[512 * k:512 * (k + 1)].rearrange(
                        "(a b) -> b a", b=128))

            # ---- main loop ----
            colpart = res.tile([128, 32], F32, name="colpart")
            dk_rk = res.tile([1, 8], F32, name="dk_rk")
            nc.vector.memset(dk_rk[:], 0.0)
            e_blks = {}
            p_row = prow.tile([1, 512], F32, tag="prow", name="p_row")
            ar1_in = dr.tile([3072], F32, name="ar1_in")
            ar1_out = dr.tile([3072], F32, name="ar1_out")
            ar2_in = dr.tile([1032], F32, name="ar2_in")
            ar2_out = dr.tile([1032], F32, name="ar2_out")

            def emit_rowmm(jb):
                nc.tensor.matmul(p_row[:], ones_bf[:], e_blks.pop(jb)[:],
                                 start=(jb == 0), stop=(jb == NJB - 1),
                                 skip_group_check=True)

            for jb in range(NJB):
                g2, joff = jb // 16, (jb % 16) * 128
                pgt = pg.tile([128, 512], F32, tag="pg", name="pg")
                for d in range(ND):
                    nc.tensor.matmul(
                        pgt[:],
                        yts[(g2, d)][:, joff:joff + 128],
                        xts[d][:],
                        start=(d == 0), stop=(d == ND - 1),
                        skip_group_check=True)
                tb = tpool.tile([128, 512], F32, tag="tb", name="tb")
                nc.vector.tensor_mul(tb[:], pgt[:], rx_b[:])
                eb = epool.tile([128, 512], BF16, tag="eb", name="eb")
                if jb < 16:
                    nc.scalar.activation(eb[:], tb[:], AF.Exp,
                                         scale=ry_scl[:, jb:jb + 1],
                                         accum_out=colpart[:, jb:jb + 1])
                else:
                    # tail blocks: column partials on the idle VectorEngine so
                    # the ACT stream is not paced by the accumulator read
                    nc.scalar.activation(eb[:], tb[:], AF.Exp,
                                         scale=ry_scl[:, jb:jb + 1])
                    nc.vector.tensor_reduce(colpart[:, jb:jb + 1], eb[:],
                                            op=ALU.add,
                                            axis=mybir.AxisListType.X)
                e_blks[jb] = eb

                if jb == 23:
                    # first 24 blocks of column partials: AllReduce mid-stream
                    nc.sync.dma_start(ar1_in[:], colpart[:, 0:24])
                    nc.gpsimd.collective_compute(
                        "AllReduce", ALU.add, replica_groups=rg,
                        ins=[ar1_in.opt()], outs=[ar1_out.opt()])

            # diag-dot chain (needed only by the final AllReduce pack)
            # diag-dot chain, off the critical AG/rx paths
            p_dd = pa.tile([1, 512], F32, tag="pa", name="p_dd")
            for d in range(ND):
                prd = tmp.tile([128, 512], BF16, tag="sq", name="prd")
                nc.vector.tensor_mul(prd[:], xts[d][:], ytos[d][:])
                nc.tensor.matmul(p_dd[:], ones_bf[:], prd[:],
                                 start=(d == 0), stop=(d == ND - 1),
                                 skip_group_check=True)
            v1 = tmp.tile([1, 512], F32, tag="v", name="v1")
            nc.vector.tensor_mul(v1[:], p_dd[:], rx[:])
            v2 = tmp.tile([1, 512], F32, tag="v", name="v2")
            nc.vector.tensor_mul(v2[:], v1[:], ryo[:])
            v3 = tmp.tile([1, 512], F32, tag="v", name="v3")
            nc.vector.tensor_scalar(v3[:], v2[:], 1.0 / TAU, None,
                                    ALU.mult, ALU.add,
                                    accum_out=dk_rk[:, 0:1])

            for jb in range(NJB):
                emit_rowmm(jb)

            # ---- row term ----
            rdv = tmp.tile([1, 512], F32, tag="v", name="rdv")
            nc.vector.tensor_scalar_add(rdv[:], p_row[:], EXTRA)
            rlnv = tmp.tile([1, 512], F32, tag="v", name="rlnv")
            nc.scalar.activation(rlnv[:], rdv[:], AF.Ln,
                                 accum_out=dk_rk[:, 1:2])

            # ---- second AllReduce: cols 2048.. + the two scalars ----
            nc.sync.dma_start(ar2_in[0:1024], colpart[:, 24:32])
            nc.sync.dma_start(ar2_in[1024:1032], dk_rk[:])
            nc.gpsimd.collective_compute(
                "AllReduce", ALU.add, replica_groups=rg,
                ins=[ar2_in.opt()], outs=[ar2_out.opt()])

            # ---- col term + final scalar (replicated on every core) ----
            csum1 = tmp.tile([128, 24], F32, tag="w", name="csum1")
            nc.sync.dma_start(csum1[:], ar1_out[:])
            cd1 = tmp.tile([128, 24], F32, tag="w", name="cd1")
            nc.vector.tensor_scalar_add(cd1[:], csum1[:], EXTRA)
            cln1 = tmp.tile([128, 24], F32, tag="w", name="cln1")
            cacc = res.tile([128, 2], F32, name="cacc")
            nc.scalar.activation(cln1[:], cd1[:], AF.Ln,
                                 accum_out=cacc[:, 0:1])
            csum2 = tmp.tile([128, 8], F32, tag="w2", name="csum2")
            nc.sync.dma_start(csum2[:], ar2_out[0:1024])
            sc2 = tmp.tile([1, 2], F32, tag="s2", name="sc2", bufs=1)
            nc.sync.dma_start(sc2[:], ar2_out[1024:1026])
            cd2 = tmp.tile([128, 8], F32, tag="w2", name="cd2")
            nc.vector.tensor_scalar_add(cd2[:], csum2[:], EXTRA)
            cln2 = tmp.tile([128, 8], F32, tag="w2", name="cln2")
            nc.scalar.activation(cln2[:], cd2[:], AF.Ln,
                                 accum_out=cacc[:, 1:2])
            p_s = pa.tile([1, 1], F32, tag="pa", name="p_s")
            nc.tensor.matmul(p_s[:], ones_f[:], cacc[:, 0:1],
                             start=True, stop=False, skip_group_check=True)
            nc.tensor.matmul(p_s[:], ones_f[:], cacc[:, 1:2],
                             start=False, stop=True, skip_group_check=True)

            f1 = res.tile([1, 1], F32, name="f1")
            nc.vector.tensor_scalar_mul(f1[:], sc2[:, 0:1], 2.0)
            f2 = res.tile([1, 1], F32, name="f2")
            nc.vector.tensor_sub(f2[:], f1[:], sc2[:, 1:2])
            f3 = res.tile([1, 1], F32, name="f3")
            nc.vector.tensor_sub(f3[:], f2[:], p_s[:])
            fl = res.tile([1, 1], F32, name="fl")
            nc.vector.tensor_scalar_mul(fl[:], f3[:], COEF)
            nc.sync.dma_start(loss_out[:, :], fl[:])

    nc.compile()
    return nc


def get_nc():
    if "nc" not in _cache:
        _cache["nc"] = _build()
    return _cache["nc"]


def make_in_maps(x: np.ndarray, y: np.ndarray):
    xb = x.astype(ml_dtypes.bfloat16)
    yb = y.astype(ml_dtypes.bfloat16)
    xT = np.ascontiguousarray(xb.T)
    yT = np.ascontiguousarray(yb.T)
    in_maps = []
    for k in range(N_CORES):
        in_maps.append({
            "xT": np.ascontiguousarray(xT[:, k * BL:(k + 1) * BL]),
            "yT": yT,
            "yTown": np.ascontiguousarray(yT[:, k * BL:(k + 1) * BL]),
        })
    return in_maps


def kernel(x: np.ndarray, y: np.ndarray) -> np.ndarray:
    nc = get_nc()
    in_maps = make_in_maps(np.asarray(x), np.asarray(y))
    res = run_bass_kernel_spmd(nc, in_maps, core_ids=list(range(N_CORES)))
    loss = res.results[0]["loss"]
    return np.asarray(loss, dtype=np.float32).reshape(())



# revision 8
# speedup vs baseline: 1.0041x; 1.0041x over previous
"""Contrastive (InfoNCE-style symmetric) loss on 8 trn2 NeuronCores.

Reference math (B=4096, D=1024, fp32):
    xn = x / max(||x_i||, eps);  yn = y / max(||y_j||, eps)
    S[i,j] = xn_i . yn_j ;  E = exp(S/tau)
    extra = B*eps + eps
    row_denom_i = sum_j E[i,j] + extra ; col_denom_j = sum_i E[i,j] + extra
    loss = -1/(2B) * ( 2*sum_i S_ii/tau - sum_i ln(row_denom_i)
                       - sum_j ln(col_denom_j) )

Design: batch dim of x sharded (512 local i per core); every core holds the
full y. All operands fp8e4m3; the [4096, 512] block of S^T is computed with
DoubleRow matmuls (256-deep contraction per MM, 128 MMs instead of 256).
x is pre-scaled on device by 16/||x_i|| (the 16 escapes fp8 subnormals and
cancels against the 1/(16*tau*||y_j||) exp scale), so the ScalarE drains
PSUM directly: eb = Exp(psum * ry_j) with the per-partition scale AP, and
the same op emits column-partial sums via accum_out. Row denominators
accumulate on TensorE as ones^T @ E, interleaved 2 blocks behind the exp
stream. y-norms are computed locally per 512-column slab (squares on DVE,
ones^T DoubleRow matmul, then a PSUM->DRAM->[128,4] rearrange and a
Newton-rsqrt chain on DVE) - no AllGather and no ACT table switches; the
Activation engine holds the Exp table for the whole kernel except one Ln
load at the tail. A tiny AllReduce issued at t=0 absorbs the one-time
collective entry latency; a single final AllReduce carries the column
partials plus the diag/row scalars.
"""
import numpy as np
import ml_dtypes

import concourse.bacc as bacc
import concourse.mybir as mybir
import concourse.tile as tile
from concourse.bass_utils import run_bass_kernel_spmd

AF = mybir.ActivationFunctionType
ALU = mybir.AluOpType
PM = mybir.MatmulPerfMode
FP8 = mybir.dt.float8e4
BF16 = mybir.dt.bfloat16
F32 = mybir.dt.float32
I32 = mybir.dt.int32

B = 4096
D = 1024
N_CORES = 8
BL = B // N_CORES          # 512 local x rows
TAU = 0.07
EPS = 1e-6
EXTRA = B * EPS + EPS
COEF = -1.0 / (2.0 * B)

XS = 16.0                  # fp8 subnormal-escape scale folded into x
RY_C = 1.0 / (XS * TAU)    # folded into the exp scale
NSLAB = 8                  # 512-j-column slabs
NJB = 32                   # 128-j blocks (4 per slab)
N_WARM0 = 10               # PE warm-up while input DMAs fly
N_WARM1 = 12               # bridge until xn is ready
RSQRT_SEED = 0x5F3759DF

_cache: dict = {}


def _newton_rsqrt(nc, pool, q, out, final_scale):
    """out = final_scale * rsqrt(q) via bit-trick seed + 2 NR iterations.
    q, out: [128, 4] f32 APs. Emits 10 DVE ops."""
    sh = list(q.shape)
    sd = pool.tile(sh, F32, tag="nw", name="nw_sd")
    nc.vector.tensor_scalar(sd[:].bitcast(I32), q.bitcast(I32),
                            1, None, ALU.arith_shift_right)
    nc.vector.tensor_scalar(sd[:].bitcast(I32), sd[:].bitcast(I32),
                            RSQRT_SEED, -1, ALU.subtract, ALU.mult)
    r = sd
    for it in range(2):
        r2 = pool.tile(sh, F32, tag="nw", name="nw_r2")
        nc.vector.tensor_mul(r2[:], r[:], r[:])
        h = pool.tile(sh, F32, tag="nw", name="nw_h")
        nc.vector.tensor_mul(h[:], r2[:], q)
        h2 = pool.tile(sh, F32, tag="nw", name="nw_h2")
        nc.vector.tensor_scalar(h2[:], h[:], -0.5, 1.5, ALU.mult, ALU.add)
        if it == 0:
            rn = pool.tile(sh, F32, tag="nw", name="nw_rn")
            nc.vector.tensor_mul(rn[:], r[:], h2[:])
            r = rn
        else:
            nc.vector.scalar_tensor_tensor(out, r[:], float(final_scale),
                                           h2[:], ALU.mult, ALU.mult)


def _build():
    nc = bacc.Bacc("TRN2", target_bir_lowering=False, debug=False,
                   num_devices=N_CORES)

    x2 = nc.dram_tensor("x2", [128, 2, 4, BL], FP8, kind="ExternalInput")
    y2 = nc.dram_tensor("y2", [NSLAB, 128, 2, 4, 512], FP8,
                        kind="ExternalInput")
    yo2 = nc.dram_tensor("yo2", [128, 2, 4, BL], FP8, kind="ExternalInput")
    loss_out = nc.dram_tensor("loss", [1, 1], F32, kind="ExternalOutput")

    rg = [list(range(N_CORES))]

    with tile.TileContext(nc) as tc:
        with (
            tc.tile_pool(name="res", bufs=1) as res,
            tc.tile_pool(name="ypool", bufs=1) as ypool,
            tc.tile_pool(name="sqp", bufs=3) as sqp,
            tc.tile_pool(name="rows", bufs=3) as rows,
            tc.tile_pool(name="nw", bufs=14) as nw,
            tc.tile_pool(name="eblk", bufs=5) as epool,
            tc.tile_pool(name="pg", bufs=4, space="PSUM") as pg,
            tc.tile_pool(name="pa", bufs=2, space="PSUM") as pa,
            tc.tile_pool(name="prow", bufs=1, space="PSUM") as prow,
            tc.tile_pool(name="dram", bufs=1, space="DRAM") as dr,
        ):
            # ---- t=0: dummy collective (absorbs entry barrier), ACT table
            # load, PE warm-up, input DMAs ----
            dums = res.tile([1, 8], F32, name="dums")
            nc.vector.memset(dums[:], 1.0)
            dum_in = dr.tile([8], F32, name="dum_in")
            dum_out = dr.tile([8], F32, name="dum_out")
            nc.gpsimd.dma_start(dum_in[:], dums[:])
            nc.gpsimd.collective_compute(
                "AllReduce", ALU.add, replica_groups=rg,
                ins=[dum_in.opt()], outs=[dum_out.opt()])

            dex = res.tile([1, 1], F32, name="dex")
            nc.vector.memset(dex[:], 0.5)
            dexo = res.tile([1, 1], F32, name="dexo")
            nc.scalar.activation(dexo[:], dex[:], AF.Exp)

            wsrc = res.tile([128, 512], BF16, name="wsrc")
            nc.vector.memset(wsrc[:], 0.125)
            wp = pg.tile([128, 512], F32, tag="pg", name="wp")
            for _ in range(N_WARM0):
                nc.tensor.matmul(wp[:], wsrc[:, 0:128], wsrc[:],
                                 start=True, stop=True, skip_group_check=True)

            xt2 = res.tile([128, 2, 4, BL], FP8, name="xt2")
            nc.sync.dma_start(xt2[:], x2[:, :, :, :])
            ys = []
            for s in range(NSLAB):
                t = ypool.tile([128, 2, 4, 512], FP8, tag=f"ys{s}",
                               name=f"ys{s}")
                nc.sync.dma_start(t[:], y2[s, :, :, :, :])
                ys.append(t)
            yot = res.tile([128, 2, 4, BL], FP8, name="yot")
            nc.sync.dma_start(yot[:], yo2[:, :, :, :])

            ones8 = res.tile([128, 2, 16], FP8, name="ones8")
            nc.vector.memset(ones8[:], 1.0)
            ones_bf = res.tile([128, 1], BF16, name="ones_bf")
            nc.vector.memset(ones_bf[:], 1.0)
            ones_f = res.tile([128, 1], F32, name="ones_f")
            nc.vector.memset(ones_f[:], 1.0)
            extra_t = res.tile([128, 1], F32, name="extra_t")
            nc.vector.memset(extra_t[:], EXTRA)

            # ---- x norms -> rx4 = 16*rsqrt(||x||^2) -> broadcast rx_b ----
            p_nx = pa.tile([1, 512], F32, tag="pa", name="p_nx")
            for c in range(4):
                sq = sqp.tile([128, 2, 512], FP8, tag="sq", name=f"sqx{c}")
                nc.vector.tensor_mul(sq[:], xt2[:, :, c:c + 1, :],
                                     xt2[:, :, c:c + 1, :])
                nc.tensor.matmul(p_nx[:], ones8[:, :, 0:1], sq[:],
                                 start=(c == 0), stop=(c == 3),
                                 perf_mode=PM.DoubleRow, skip_group_check=True)
            nxrow = rows.tile([1, 512], F32, tag="row", name="nxrow")
            nc.vector.tensor_copy(nxrow[:], p_nx[:])
            nx_d = dr.tile([512], F32, name="nx_d")
            nc.sync.dma_start(nx_d[:], nxrow[:])
            nx4 = res.tile([128, 4], F32, name="nx4")
            nc.sync.dma_start(nx4[:],
                                nx_d[:].rearrange("(a b) -> b a", b=128))
            rx4 = res.tile([128, 4], F32, name="rx4")
            _newton_rsqrt(nc, nw, nx4[:], rx4[:], XS)
            rx_d = dr.tile([512], F32, name="rx_d")
            nc.sync.dma_start(rx_d[:].rearrange("(a b) -> b a", b=128),
                                rx4[:])
            rx_b = res.tile([128, 512], F32, name="rx_b")
            nc.sync.dma_start(
                rx_b[:],
                rx_d[:].rearrange("(o a) -> o a", o=1).broadcast_to([128, 512]))

            # bridge warm-up while the rx chain completes
            for _ in range(N_WARM1):
                nc.tensor.matmul(wp[:], wsrc[:, 0:128], wsrc[:],
                                 start=True, stop=True, skip_group_check=True)

            # ---- xn = fp8(x * rx_b) ----
            xn = []
            for c in range(4):
                t = res.tile([128, 2, 512], FP8, name=f"xn{c}")
                for i in range(2):
                    nc.vector.tensor_mul(t[:, i:i + 1, :],
                                         xt2[:, i:i + 1, c:c + 1, :],
                                         rx_b[:])
                xn.append(t)

            # ---- main slab loop ----
            colpart = res.tile([128, 32], F32, name="colpart")
            ry_all = res.tile([128, 32], F32, name="ry_all")
            dk_rk = res.tile([1, 8], F32, name="dk_rk")
            nc.vector.memset(dk_rk[:], 0.0)
            ny_d = dr.tile([NSLAB, 512], F32, name="ny_d")
            p_row = prow.tile([1, 512], F32, tag="prow", name="p_row")
            e_blks = {}

            def emit_rowmm(g):
                nc.tensor.matmul(p_row[:], ones_bf[:], e_blks.pop(g)[:],
                                 start=(g == 0), stop=(g == NJB - 1),
                                 skip_group_check=True)

            for s in range(NSLAB):
                # slab norms: ny2 = sum_d y^2 -> dance -> newton -> ry cols
                p_ny = pa.tile([1, 512], F32, tag="pa", name=f"p_ny{s}")
                for c in range(4):
                    sq = sqp.tile([128, 2, 512], FP8, tag="sq",
                                  name=f"sqy{s}_{c}")
                    nc.vector.tensor_mul(sq[:], ys[s][:, :, c:c + 1, :],
                                         ys[s][:, :, c:c + 1, :])
                    nc.tensor.matmul(p_ny[:], ones8[:, :, 0:1], sq[:],
                                     start=(c == 0), stop=(c == 3),
                                     perf_mode=PM.DoubleRow,
                                     skip_group_check=True)
                nyrow = rows.tile([1, 512], F32, tag="row", name=f"nyrow{s}")
                nc.vector.tensor_copy(nyrow[:], p_ny[:])
                nc.sync.dma_start(ny_d[s, :], nyrow[:])
                ny4 = nw.tile([128, 4], F32, tag="ny4", name=f"ny4_{s}")
                nc.sync.dma_start(
                    ny4[:], ny_d[s, :].rearrange("(a b) -> b a", b=128))
                _newton_rsqrt(nc, nw, ny4[:], ry_all[:, 4 * s:4 * s + 4],
                              RY_C)

                # main blocks
                for a in range(4):
                    g = 4 * s + a
                    pgt = pg.tile([128, 512], F32, tag="pg", name="pgt")
                    for c in range(4):
                        nc.tensor.matmul(
                            pgt[:],
                            ys[s][:, :, c:c + 1, a * 128:(a + 1) * 128],
                            xn[c][:],
                            start=(c == 0), stop=(c == 3),
                            perf_mode=PM.DoubleRow, skip_group_check=True)
                    eb = epool.tile([128, 512], BF16, tag="eb", name="eb")
                    nc.scalar.activation(eb[:], pgt[:], AF.Exp,
                                         scale=ry_all[:, g:g + 1],
                                         accum_out=colpart[:, g:g + 1])
                    e_blks[g] = eb
                    if g >= 2:
                        emit_rowmm(g - 2)

                if s == 3:
                    # own-y norms + diag chain, mid-stream off the tail
                    p_nyo = pa.tile([1, 512], F32, tag="pa", name="p_nyo")
                    for c in range(4):
                        sq = sqp.tile([128, 2, 512], FP8, tag="sq",
                                      name=f"sqo{c}")
                        nc.vector.tensor_mul(sq[:], yot[:, :, c:c + 1, :],
                                             yot[:, :, c:c + 1, :])
                        nc.tensor.matmul(p_nyo[:], ones8[:, :, 0:1], sq[:],
                                         start=(c == 0), stop=(c == 3),
                                         perf_mode=PM.DoubleRow,
                                         skip_group_check=True)
                    nyorow = rows.tile([1, 512], F32, tag="row",
                                       name="nyorow")
                    nc.vector.tensor_copy(nyorow[:], p_nyo[:])
                    nyo_d = dr.tile([512], F32, name="nyo_d")
                    nc.sync.dma_start(nyo_d[:], nyorow[:])
                    nyo4 = res.tile([128, 4], F32, name="nyo4")
                    nc.sync.dma_start(
                        nyo4[:], nyo_d[:].rearrange("(a b) -> b a", b=128))
                    ryo4 = res.tile([128, 4], F32, name="ryo4")
                    _newton_rsqrt(nc, nw, nyo4[:], ryo4[:], RY_C)

                if s == 4:
                    p_dd = pa.tile([1, 512], F32, tag="pa", name="p_dd")
                    for c in range(4):
                        prd = sqp.tile([128, 2, 512], FP8, tag="sq",
                                       name=f"prd{c}")
                        nc.vector.tensor_mul(prd[:], xt2[:, :, c:c + 1, :],
                                             yot[:, :, c:c + 1, :])
                        nc.tensor.matmul(p_dd[:], ones8[:, :, 0:1], prd[:],
                                         start=(c == 0), stop=(c == 3),
                                         perf_mode=PM.DoubleRow,
                                         skip_group_check=True)
                    ddrow = rows.tile([1, 512], F32, tag="row", name="ddrow")
                    nc.vector.tensor_copy(ddrow[:], p_dd[:])
                    dd_d = dr.tile([512], F32, name="dd_d")
                    nc.sync.dma_start(dd_d[:], ddrow[:])
                    dd4 = res.tile([128, 4], F32, name="dd4")
                    nc.sync.dma_start(
                        dd4[:], dd_d[:].rearrange("(a b) -> b a", b=128))

                if s == 5:
                    # dk = sum_i dd_i * rx_i * ryo_i  (the 16s cancel)
                    t1 = res.tile([128, 4], F32, name="dk_t1")
                    nc.vector.tensor_mul(t1[:], dd4[:], rx4[:])
                    t2 = res.tile([128, 4], F32, name="dk_t2")
                    dkacc = res.tile([128, 1], F32, name="dkacc")
                    nc.vector.scalar_tensor_tensor(t2[:], t1[:], 1.0,
                                                   ryo4[:], ALU.mult,
                                                   ALU.mult,
                                                   accum_out=dkacc[:])
                    p_dk = pa.tile([1, 1], F32, tag="pdk", name="p_dk", bufs=1)
                    nc.tensor.matmul(p_dk[:], ones_f[:], dkacc[:],
                                     start=True, stop=True,
                                     skip_group_check=True)
                    nc.vector.tensor_copy(dk_rk[:, 0:1], p_dk[:])

            emit_rowmm(NJB - 2)
            emit_rowmm(NJB - 1)

            # ---- row term: rk = sum_i ln(rowsum_i + EXTRA) ----
            rln = rows.tile([1, 512], F32, tag="row", name="rln")
            nc.scalar.activation(rln[:], p_row[:], AF.Ln,
                                 bias=extra_t[0:1, 0:1],
                                 accum_out=dk_rk[:, 1:2])

            # ---- final AllReduce: col partials + [dk, rk] ----
            ar_in = dr.tile([4104], F32, name="ar_in")
            ar_out = dr.tile([4104], F32, name="ar_out")
            nc.gpsimd.dma_start(ar_in[0:4096], colpart[:])
            nc.gpsimd.dma_start(ar_in[4096:4104], dk_rk[:])
            nc.gpsimd.collective_compute(
                "AllReduce", ALU.add, replica_groups=rg,
                ins=[ar_in.opt()], outs=[ar_out.opt()])
            car = res.tile([128, 32], F32, name="car")
            nc.gpsimd.dma_start(
                car[:], ar_out[0:4096].rearrange("(p c) -> p c", p=128))
            sc = res.tile([1, 2], F32, name="sc")
            nc.gpsimd.dma_start(sc[:], ar_out[4096:4098])

            # ---- col term + final combine (replicated on every core) ----
            cln = res.tile([128, 32], F32, name="cln")
            cacc = res.tile([128, 1], F32, name="cacc")
            nc.scalar.activation(cln[:], car[:], AF.Ln, bias=extra_t[:],
                                 accum_out=cacc[:])
            p_s = pa.tile([1, 1], F32, tag="pdk", name="p_s", bufs=1)
            nc.tensor.matmul(p_s[:], ones_f[:], cacc[:],
                             start=True, stop=True, skip_group_check=True)
            f1 = res.tile([1, 1], F32, name="f1")
            nc.vector.scalar_tensor_tensor(f1[:], sc[:, 0:1], 2.0,
                                           sc[:, 1:2], ALU.mult,
                                           ALU.subtract)
            f2 = res.tile([1, 1], F32, name="f2")
            nc.vector.tensor_sub(f2[:], f1[:], p_s[:])
            fl = res.tile([1, 1], F32, name="fl")
            nc.vector.tensor_scalar_mul(fl[:], f2[:], COEF)
            nc.sync.dma_start(loss_out[:, :], fl[:])

    nc.compile()
    return nc


def get_nc():
    if "nc" not in _cache:
        _cache["nc"] = _build()
    return _cache["nc"]


def _pack_dr(arrT: np.ndarray) -> np.ndarray:
    """[1024, J] (d-major) -> [128, 2, 4, J] with d = 256*c + 128*i + p."""
    J = arrT.shape[1]
    return np.ascontiguousarray(
        arrT.reshape(4, 2, 128, J).transpose(2, 1, 0, 3))


def make_in_maps(x: np.ndarray, y: np.ndarray):
    x8T = np.ascontiguousarray(x.astype(ml_dtypes.float8_e4m3).T)
    y8T = np.ascontiguousarray(y.astype(ml_dtypes.float8_e4m3).T)
    # y2: [slab, 128, 2, 4, 512], slab s covers j in [512s, 512s+512)
    y2 = _pack_dr(y8T).reshape(128, 2, 4, NSLAB, 512)
    y2 = np.ascontiguousarray(y2.transpose(3, 0, 1, 2, 4))
    in_maps = []
    for k in range(N_CORES):
        in_maps.append({
            "x2": _pack_dr(x8T[:, k * BL:(k + 1) * BL]),
            "y2": y2,
            "yo2": _pack_dr(y8T[:, k * BL:(k + 1) * BL]),
        })
    return in_maps


def kernel(x: np.ndarray, y: np.ndarray) -> np.ndarray:
    nc = get_nc()
    in_maps = make_in_maps(np.asarray(x), np.asarray(y))
    res = run_bass_kernel_spmd(nc, in_maps, core_ids=list(range(N_CORES)))
    loss = res.results[0]["loss"]
    return np.asarray(loss, dtype=np.float32).reshape(())
